# revision 6
# baseline (speedup 1.0000x reference)
# Grouped-GEMM "patch readout" kernel for Trainium2 (8 NeuronCores).
#
# Problem: out[b, p, :] = x[b, :, p, :].reshape(T*F) @ W[p] + bias[p]
#   x: [B=32, T=12, P=128, F=128] f32
#   W: [P=128, T*F=1536, NODES*H=768] f32   (604 MB -> the memory-bound term)
#   b: [P=128, 768] f32
#   patch_node_map: [128, 64] int  (permutation; scatter handled on host as the
#   unshard step)
#
# Sharding: expert-parallel over patches. Each of the 8 cores owns 16 patches
# and streams its 75.5 MB W slice from HBM exactly once (system-wide minimum
# traffic). Patches are processed in groups of 4, col-tiled onto the four
# 32-wide column strips of the PE array (output partitions 0/32/64/96) so the
# four M=32 matmuls run concurrently and the PE stays well under the DMA
# roofline. Per patch: 12 K-chunks of 128; stationary operand = x^T chunk
# [128, 32], moving operand = W chunk [128, 768] split 512+256 across two PSUM
# banks, accumulated over K. Bias is added during the PSUM->SBUF evacuation
# (host pre-replicates it across the batch dim).

import numpy as np

import concourse.bacc as bacc
import concourse.mybir as mybir
import concourse.tile as tile
from concourse.bass_utils import run_bass_kernel_spmd

NCORES = 8
B = 32            # batch (matmul M)
T = 12            # timesteps == K chunks of 128 (F == 128)
P = 128           # total patches
F = 128           # features == contraction per chunk
PL = P // NCORES  # 16 patches per core
N = 768           # nodes_per_patch * horizon
NODES_PER_PATCH = 64
HORIZON = 12
N_NODES = P * NODES_PER_PATCH

GRP = 4           # patches per col-tiled group
NGRP = PL // GRP  # 4 groups per core
KQ = 3            # k-chunks per W quarter-tile
NQ = T // KQ      # quarter-tiles per patch

F32 = mybir.dt.float32

_CACHE = {}


def _build_bass():
    nc = bacc.Bacc("TRN2", target_bir_lowering=False, debug=False)

    # Host-prepared layouts (see kernel()):
    #   xt   [128, PL*T*B]: xt[f, (p*T + t)*B + b] = x[b, t, p_global, f]
    #   w    [PL, T*F, N] : natural per-core W slice
    #   biasr[PL*B, N]    : bias replicated across batch, patch-major
    xt = nc.dram_tensor("xt", [F, PL * T * B], F32, kind="ExternalInput").ap()
    w = nc.dram_tensor("w", [PL, T * F, N], F32, kind="ExternalInput").ap()
    biasr = nc.dram_tensor("biasr", [PL * B, N], F32, kind="ExternalInput").ap()
    out = nc.dram_tensor("out", [PL * B, N], F32, kind="ExternalOutput").ap()

    # [PL, 128(part), T, N] view: chunk (p, t) is W[p, t*128:(t+1)*128, :]
    w4 = w.rearrange("p (t q) n -> p q t n", q=F)

    with tile.TileContext(nc) as tc:
        with (
            tc.tile_pool(name="xpool", bufs=1) as xpool,
            tc.tile_pool(name="wpool", bufs=16) as wpool,
            tc.tile_pool(name="bpool", bufs=2) as bpool,
            tc.tile_pool(name="opool", bufs=2) as opool,
            tc.tile_pool(name="ps", bufs=2, space="PSUM") as pspool,
        ):
            # x/bias/out ride the ACT HWDGE ring (nc.scalar) so the W stream
            # owns the SP ring (nc.sync) end to end — HWDGE DMAs are FIFO per
            # issuing engine, so mixing them would stall the W stream.
            x_sb = xpool.tile([F, PL * T * B], F32)
            half = PL * T * B // 2
            nc.scalar.dma_start(x_sb[:, :half], xt[:, :half])
            nc.scalar.dma_start(x_sb[:, half:], xt[:, half:])

            for g in range(NGRP):
                # stream this group's W as 16 quarter-K tiles (4 patches x 4)
                w_sb = {}
                for q in range(NQ):
                    for j in range(GRP):
                        p = g * GRP + j
                        wt = wpool.tile([F, KQ * N], F32, tag="w")
                        nc.sync.dma_start(
                            wt[:].rearrange("f (t n) -> f t n", n=N),
                            w4[p, :, q * KQ : (q + 1) * KQ],
                        )
                        w_sb[(q, j)] = wt

                bias_sb = bpool.tile([GRP * B, N], F32)
                nc.scalar.dma_start(
                    bias_sb[:], biasr[g * GRP * B : (g + 1) * GRP * B]
                )

                ps = pspool.tile([GRP * B, N], F32)
                for t in range(T):
                    q, kk = divmod(t, KQ)
                    for n0, n1 in ((0, 512), (512, N)):
                        for j in range(GRP):
                            p = g * GRP + j
                            lhsT = x_sb[:, (p * T + t) * B : (p * T + t + 1) * B]
                            # out partition offset 32*j => col-tile strip j
                            nc.tensor.matmul(
                                ps[j * B : (j + 1) * B, n0:n1],
                                lhsT,
                                w_sb[(q, j)][:, kk * N + n0 : kk * N + n1],
                                start=(t == 0),
                                stop=(t == T - 1),
                                tile_position=(0, j * B),
                            )

                o_sb = opool.tile([GRP * B, N], F32)
                nc.vector.tensor_tensor(
                    out=o_sb[:], in0=ps[:], in1=bias_sb[:], op=mybir.AluOpType.add
                )
                nc.scalar.dma_start(out[g * GRP * B : (g + 1) * GRP * B], o_sb[:])

    nc.finalize()
    return nc


def _get_nc():
    if "nc" not in _CACHE:
        _CACHE["nc"] = _build_bass()
    return _CACHE["nc"]


def _make_in_maps(x, W, b):
    x = np.asarray(x, dtype=np.float32)
    W = np.asarray(W, dtype=np.float32)
    b = np.asarray(b, dtype=np.float32)
    # [f, p, t, b] so each per-core slice reshapes to the SBUF layout directly
    xt_full = np.ascontiguousarray(np.transpose(x, (3, 2, 1, 0)))
    in_maps = []
    for c in range(NCORES):
        p0 = c * PL
        xt = np.ascontiguousarray(xt_full[:, p0 : p0 + PL]).reshape(F, PL * T * B)
        biasr = np.ascontiguousarray(
            np.broadcast_to(b[p0 : p0 + PL, None, :], (PL, B, N))
        ).reshape(PL * B, N)
        in_maps.append({"xt": xt, "w": W[p0 : p0 + PL], "biasr": biasr})
    return in_maps


def _unshard(results, patch_node_map):
    # results[c]["out"]: [PL*B, N] -> global [B, N_NODES, HORIZON] scatter
    out_pbn = np.concatenate(
        [np.asarray(r["out"]).reshape(PL, B, N) for r in results], axis=0
    )
    src = (
        out_pbn.reshape(P, B, NODES_PER_PATCH, HORIZON)
        .transpose(1, 0, 2, 3)
        .reshape(B, N_NODES, HORIZON)
    )
    idx = np.asarray(patch_node_map).reshape(-1).astype(np.int64)
    out_all = np.empty((B, N_NODES, HORIZON), dtype=np.float32)
    out_all[:, idx, :] = src
    return out_all


def run(x, W, b, patch_node_map, trace=False):
    nc = _get_nc()
    in_maps = _make_in_maps(x, W, b)
    res = run_bass_kernel_spmd(
        nc, in_maps, core_ids=list(range(NCORES)), trace=trace
    )
    out_all = _unshard(res.results, patch_node_map)
    return out_all, res


def kernel(x, W, b, patch_node_map):
    out_all, _ = run(x, W, b, patch_node_map)
    return out_all


# revision 7
# speedup vs baseline: 1.1511x; 1.1511x over previous
# Grouped-GEMM "patch readout" kernel for Trainium2 (8 NeuronCores).
#
# Problem: out[b, p, :] = x[b, :, p, :].reshape(T*F) @ W[p] + bias[p]
#   x: [B=32, T=12, P=128, F=128] f32
#   W: [P=128, T*F=1536, NODES*H=768] f32   (604 MB -> the memory-bound term)
#   b: [P=128, 768] f32
#   patch_node_map: [128, 64] int  (permutation; scatter handled on host as the
#   unshard step)
#
# Sharding: expert-parallel over patches. Each of the 8 cores owns 16 patches
# and streams its 75.5 MB W slice from HBM exactly once (system-wide minimum
# traffic). Patches are processed in groups of 4, col-tiled onto the four
# 32-wide column strips of the PE array (output partitions 0/32/64/96) so the
# four M=32 matmuls run concurrently and the PE stays well under the DMA
# roofline. Per patch: 12 K-chunks of 128; stationary operand = x^T chunk
# [128, 32], moving operand = W chunk [128, 768] split 512+256 across two PSUM
# banks, accumulated over K. Bias is added during the PSUM->SBUF evacuation
# (host pre-replicates it across the batch dim). W rides the SP HWDGE ring
# exclusively (HWDGE is FIFO per issuing engine); x loads go via the ACT ring.
# The final group streams W in finer slices so the end-of-stream PE tail is
# short.

import numpy as np

import concourse.bacc as bacc
import concourse.mybir as mybir
import concourse.tile as tile
from concourse.bass_utils import run_bass_kernel_spmd

NCORES = 8
B = 32            # batch (matmul M)
T = 12            # timesteps == K chunks of 128 (F == 128)
P = 128           # total patches
F = 128           # features == contraction per chunk
PL = P // NCORES  # 16 patches per core
N = 768           # nodes_per_patch * horizon
NODES_PER_PATCH = 64
HORIZON = 12
N_NODES = P * NODES_PER_PATCH

GRP = 4           # patches per col-tiled group
NGRP = PL // GRP  # 4 groups per core
KH = T // 2       # k-chunks per W half-tile (steady-state groups)
KS = 2            # k-chunks per W slice in the final group

F32 = mybir.dt.float32

_CACHE = {}


def _build_bass():
    nc = bacc.Bacc("TRN2", target_bir_lowering=False, debug=False)

    # Host-prepared layouts (see kernel()):
    #   xt   [128, PL*T*B]: xt[f, (p*T + t)*B + b] = x[b, t, p_global, f]
    #   w    [PL, T*F, N] : natural per-core W slice
    #   biasr[PL*B, N]    : bias replicated across batch, patch-major
    xt = nc.dram_tensor("xt", [F, PL * T * B], F32, kind="ExternalInput").ap()
    w = nc.dram_tensor("w", [PL, T * F, N], F32, kind="ExternalInput").ap()
    biasr = nc.dram_tensor("biasr", [PL * B, N], F32, kind="ExternalInput").ap()
    out = nc.dram_tensor("out", [PL * B, N], F32, kind="ExternalOutput").ap()

    # [PL, 128(part), T, N] view: chunk (p, t) is W[p, t*128:(t+1)*128, :]
    w4 = w.rearrange("p (t q) n -> p q t n", q=F)

    with tile.TileContext(nc) as tc:
        with (
            tc.tile_pool(name="xpool", bufs=1) as xpool,
            tc.tile_pool(name="wpool", bufs=8) as wpool,
            tc.tile_pool(name="bpool", bufs=2) as bpool,
            tc.tile_pool(name="opool", bufs=2) as opool,
            tc.tile_pool(name="ps", bufs=2, space="PSUM") as pspool,
        ):
            x_sb = xpool.tile([F, PL * T * B], F32)
            half = PL * T * B // 2
            nc.scalar.dma_start(x_sb[:, :half], xt[:, :half])
            nc.scalar.dma_start(x_sb[:, half:], xt[:, half:])

            for g in range(NGRP):
                last = g == NGRP - 1
                # k-chunks per W slice: halves mid-kernel, finer at the end
                ks = KS if last else KH
                nslice = T // ks
                w_sb = {}
                for s in range(nslice):
                    for j in range(GRP):
                        p = g * GRP + j
                        wt = wpool.tile([F, ks * N], F32, tag="w")
                        nc.sync.dma_start(
                            wt[:].rearrange("f (t n) -> f t n", n=N),
                            w4[p, :, s * ks : (s + 1) * ks],
                        )
                        for kk in range(ks):
                            w_sb[(s * ks + kk, j)] = wt[:, kk * N : (kk + 1) * N]

                bias_sb = bpool.tile([GRP * B, N], F32)
                nc.scalar.dma_start(
                    bias_sb[:], biasr[g * GRP * B : (g + 1) * GRP * B]
                )

                ps = pspool.tile([GRP * B, N], F32)
                for t in range(T):
                    for n0, n1 in ((0, 512), (512, N)):
                        for j in range(GRP):
                            p = g * GRP + j
                            lhsT = x_sb[:, (p * T + t) * B : (p * T + t + 1) * B]
                            # out partition offset 32*j => col-tile strip j
                            nc.tensor.matmul(
                                ps[j * B : (j + 1) * B, n0:n1],
                                lhsT,
                                w_sb[(t, j)][:, n0:n1],
                                start=(t == 0),
                                stop=(t == T - 1),
                                tile_position=(0, j * B),
                            )

                o_sb = opool.tile([GRP * B, N], F32)
                nc.vector.tensor_tensor(
                    out=o_sb[:], in0=ps[:], in1=bias_sb[:], op=mybir.AluOpType.add
                )
                nc.scalar.dma_start(out[g * GRP * B : (g + 1) * GRP * B], o_sb[:])

    nc.finalize()
    return nc


def _get_nc():
    if "nc" not in _CACHE:
        _CACHE["nc"] = _build_bass()
    return _CACHE["nc"]


def _make_in_maps(x, W, b):
    x = np.asarray(x, dtype=np.float32)
    W = np.asarray(W, dtype=np.float32)
    b = np.asarray(b, dtype=np.float32)
    # [f, p, t, b] so each per-core slice reshapes to the SBUF layout directly
    xt_full = np.ascontiguousarray(np.transpose(x, (3, 2, 1, 0)))
    in_maps = []
    for c in range(NCORES):
        p0 = c * PL
        xt = np.ascontiguousarray(xt_full[:, p0 : p0 + PL]).reshape(F, PL * T * B)
        biasr = np.ascontiguousarray(
            np.broadcast_to(b[p0 : p0 + PL, None, :], (PL, B, N))
        ).reshape(PL * B, N)
        in_maps.append({"xt": xt, "w": W[p0 : p0 + PL], "biasr": biasr})
    return in_maps


def _unshard(results, patch_node_map):
    # results[c]["out"]: [PL*B, N] -> global [B, N_NODES, HORIZON] scatter
    out_pbn = np.concatenate(
        [np.asarray(r["out"]).reshape(PL, B, N) for r in results], axis=0
    )
    src = (
        out_pbn.reshape(P, B, NODES_PER_PATCH, HORIZON)
        .transpose(1, 0, 2, 3)
        .reshape(B, N_NODES, HORIZON)
    )
    idx = np.asarray(patch_node_map).reshape(-1).astype(np.int64)
    out_all = np.empty((B, N_NODES, HORIZON), dtype=np.float32)
    out_all[:, idx, :] = src
    return out_all


def run(x, W, b, patch_node_map, trace=False):
    nc = _get_nc()
    in_maps = _make_in_maps(x, W, b)
    res = run_bass_kernel_spmd(
        nc, in_maps, core_ids=list(range(NCORES)), trace=trace
    )
    out_all = _unshard(res.results, patch_node_map)
    return out_all, res


def kernel(x, W, b, patch_node_map):
    out_all, _ = run(x, W, b, patch_node_map)
    return out_all
